# revision 11
# baseline (speedup 1.0000x reference)
"""Trainium2 Bass kernel for nn_ConditionalChannelProjection (v2, bf16).

Reference computation (per sample b):
    mod = silu(emb) @ ada_w.T + ada_b          -> shift (C,), scale (C,)
    rms = rsqrt(mean_c(x^2) + eps)             -> per-pixel over channels
    xm  = (x * rms) * (1 + scale) + shift
    y   = selu(conv_w @ xm + conv_b)           (1x1 conv == channel GEMM)

v2 design (per 1024-pixel unit, bf16 data paths, fp32 PSUM):
    xsq  = x*x                                  (GPSIMD/DVE pixel-split)
    pm   = ones(1/C)-matmul(xsq)                (PE -> PSUM, bcast over parts)
    rmsb = rsqrt~(pm)                           (custom DVE op: linear seed +
                                                 1 Newton step, single pass;
                                                 partially ACT ln/exp as a
                                                 load-balance knob)
    xn   = x * rmsb                             (DVE TT bf16 2x)
    pg   = W''-gemm(xn) + lam*s (K=1 bias row)  (PE, bf16, fp32 PSUM)
      where W''[c,o] = lam*(1+scale_c)*w[o,c],  s[o] = w@shift + conv_b
    es   = Exp(pg/lam + ln(lam*alpha))          (ACT, one op per o-pair)
    y    = min(es, lam*a) + max(pg - lam*a, -lam*a)   (custom DVE op, 1 pass)
      == selu(u)*...: pg = lam*u, es = lam*a*e^u; exact selu identity.

Sharding: data-parallel over batch, 4 samples per core, params replicated.
I/O in bf16 (host converts); rel-err budget 2e-2, measured ~4e-3.
"""

import numpy as np

import concourse.bass as bass
import concourse.bacc as bacc
import concourse.tile as tile
import concourse.mybir as mybir
from concourse import bass_utils
from concourse.masks import make_identity
from concourse.alu_op_type import AluOpType as Op

# ---------------------------------------------------------------------------
# ACT table pinning: both Exp and Ln resolve to natural_log_exp_and_others so
# a single ACT_TABLE_LOAD is emitted (the greedy insertion pass would
# otherwise thrash between exp_and_others and natural_log sets).
import concourse.bacc as _bacc_mod
import concourse.hw_specs as _hw_specs

_ORIG_GET_TABLES = _hw_specs.get_activation_tables
_KEEP_TABLE = "natural_log_exp_and_others"


def _patched_get_tables(arch):
    tables = _ORIG_GET_TABLES(arch)
    return {name: (funcs if name == _KEEP_TABLE else set())
            for name, funcs in tables.items()}


_bacc_mod.get_activation_tables = _patched_get_tables

# ---------------------------------------------------------------------------
# Custom DVE ops (registered into concourse.dve_ops' module-level registry,
# which bass_utils/bass2jax read by object identity).
from concourse import dve_ops as _dve_ops_mod
from concourse.dve_ops import DveOp, get_dve_sub_opcode
from concourse.dve_spec import (
    C0, C1, C2, Spec, Src0, Src1, lower, _has_src1, maxx, minn, sq,
)
from concourse.dve_uop import DveOpSpec


def _register_dve_op(name, spec, subdim=False):
    for existing in _dve_ops_mod.OPS:
        if existing.name == name:
            return existing
    row = max(_dve_ops_mod._SUB_OPCODE_FOR_NAME.values()) + 1
    assert row < 0x20, "custom-DVE opcode rows exhausted"
    _dve_ops_mod._SUB_OPCODE_FOR_NAME[name] = row
    shas = {}
    for ver in ("v3", "v4"):
        try:
            s = DveOpSpec(name=name, opcode=row, uops=lower(spec, ver=ver),
                          rd1_en=_has_src1(spec))
            shas[ver] = s.sha(ver)
        except Exception:
            pass
    op = DveOp(name, spec, subdim=subdim, uops_sha=shas)
    _dve_ops_mod.OPS.append(op)
    _dve_ops_mod.CUSTOM_DVE_SPECS[name] = op.spec
    return op


# rsqrt(m) ~= z*(3 - m*(2z)^2), z = C0 - C1*m: distribution-weighted linear
# seed + one Newton-Raphson step. Fit on the empirical mean-square range of
# randn data (m in [0.55, 2.3], bulk near 1): rms rel err 4.5e-4.
_RSQ_A2 = 0.69984783  # a/2
_RSQ_B2 = 0.19598216  # b/2
_z = C0 - C1 * Src0
RSQRT_SEED_NR = _register_dve_op(
    "RSQRT_SEED_NR_ANT",
    Spec(
        body=_z * (C2 - Src0 * sq(_z + _z)),
        reference=lambda in0, in1, c0, c1, c2:
            (c0 - c1 * in0) * (c2 - in0 * ((c0 - c1 * in0) * 2.0) ** 2),
    ),
)

# selu combine: out = min(es, imm2) + max(pg + s0, s1)
# with s0 = -lam*alpha, s1 = -lam*alpha, imm2 = lam*alpha.
SELU_COMBINE = _register_dve_op(
    "SELU_COMBINE_ANT",
    Spec(
        body=minn(Src0, C2) + maxx(Src1 + C0, C1),
        reference=lambda in0, in1, s0, s1, imm2:
            np.minimum(in0, imm2) + np.maximum(in1 + s0, s1),
    ),
)

AF = mybir.ActivationFunctionType
F32 = mybir.dt.float32
BF16 = mybir.dt.bfloat16

B, C, H, W, E = 32, 256, 64, 64, 1024
HW = H * W                    # 4096 pixels per sample
NCORES = 8
BL = B // NCORES              # 4 samples per core
PW = 1024                     # pixels per unit
NU = HW // PW                 # 4 units per sample
KT = C // 128                 # 2 channel tiles
OT = C // 128                 # 2 output tiles
EPS = 1e-6

LAM = 1.0507009873554804934193349852946
ALPHA = 1.6732632423543772848170429916717
LA = LAM * ALPHA
LN_LA = float(np.log(LA))

# load-balance knobs
GPX = 832      # pixels per ktile of x^2 done on GPSIMD (rest on DVE)
RMSD = 0       # pixels of rms via custom-DVE rsqrt (rest via ACT ln+exp)


def _build_program(reps=1):
    nc = bacc.Bacc("TRN2", target_bir_lowering=False, debug=False,
                   num_devices=NCORES)

    x_d = nc.dram_tensor("x", (BL, C, HW), BF16, kind="ExternalInput")
    emb_d = nc.dram_tensor("emb", (BL, E), F32, kind="ExternalInput")
    adaw_d = nc.dram_tensor("ada_w", (2 * C, E), F32, kind="ExternalInput")
    adab_d = nc.dram_tensor("ada_b", (2 * C,), F32, kind="ExternalInput")
    convw_d = nc.dram_tensor("conv_w", (C, C), F32, kind="ExternalInput")
    convb_d = nc.dram_tensor("conv_b", (C,), F32, kind="ExternalInput")
    y_d = nc.dram_tensor("y", (BL, C, HW), BF16, kind="ExternalOutput")

    with tile.TileContext(nc) as tc:
        for _ in range(reps):
            _kernel(nc, tc, x_d, emb_d, adaw_d, adab_d, convw_d, convb_d, y_d)

    nc.compile()
    return nc


def _kernel(nc, tc, x_d, emb_d, adaw_d, adab_d, convw_d, convb_d, y_d):
    from contextlib import ExitStack
    ctx = ExitStack()
    with ctx:
        consts = ctx.enter_context(tc.tile_pool(name="consts", bufs=1))
        params = ctx.enter_context(tc.tile_pool(name="params", bufs=1))
        spool = ctx.enter_context(tc.tile_pool(name="spool", bufs=2))
        xpool = ctx.enter_context(tc.tile_pool(name="xpool", bufs=3))
        qpool = ctx.enter_context(tc.tile_pool(name="qpool", bufs=2))
        npool = ctx.enter_context(tc.tile_pool(name="npool", bufs=2))
        rpool = ctx.enter_context(tc.tile_pool(name="rpool", bufs=2))
        epool = ctx.enter_context(tc.tile_pool(name="epool", bufs=2))
        ypool = ctx.enter_context(tc.tile_pool(name="ypool", bufs=3))
        wpool = ctx.enter_context(tc.tile_pool(name="wpool", bufs=2))

        # ---- constants -------------------------------------------------
        ident = consts.tile([128, 128], F32)
        make_identity(nc, ident[:])
        onesC = consts.tile([128, 128], BF16)
        nc.gpsimd.memset(onesC[:], 1.0 / C)
        ones_row = consts.tile([1, 512], BF16)
        nc.gpsimd.memset(ones_row[:], 1.0)
        zero_col = consts.tile([128, 1], F32)
        nc.gpsimd.memset(zero_col[:], 0.0)
        eps_col = consts.tile([128, 1], F32)
        nc.gpsimd.memset(eps_col[:], EPS)
        lnla_col = consts.tile([128, 1], F32)
        nc.gpsimd.memset(lnla_col[:], LN_LA)

        # ---- parameter prep (runs once, overlapped with first x DMA) ---
        with tc.tile_pool(name="prep", bufs=1) as prep, \
             tc.tile_pool(name="psum_p", bufs=2, space="PSUM") as psum_p:

            # conv_w -> wT[c_part, k_tile, o]  (transposed via PE, fp32)
            cw = prep.tile([128, KT, C], F32)
            nc.sync.dma_start(
                cw[:], convw_d.ap().rearrange("(ot op) c -> op ot c", op=128))
            wT = params.tile([128, KT, C], F32)
            for k in range(KT):
                for m in range(KT):
                    pt = psum_p.tile([128, 128], F32, tag="pp")
                    nc.tensor.transpose(
                        pt[:], cw[:, m, k * 128:(k + 1) * 128], ident[:])
                    nc.vector.tensor_copy(wT[:, k, m * 128:(m + 1) * 128],
                                          pt[:])

            # silu(emb) via exp/reciprocal (keeps ACT table = ln/exp set)
            embt = prep.tile([BL, E], F32)
            nc.sync.dma_start(embt[:], emb_d.ap())
            sig = prep.tile([BL, E], F32)
            nc.scalar.activation(sig[:], embt[:], AF.Exp, bias=zero_col[:BL],
                                 scale=-1.0)
            nc.vector.tensor_scalar_add(sig[:], sig[:], 1.0)
            nc.vector.reciprocal(sig[:], sig[:])
            semb = prep.tile([BL, E], F32)
            nc.vector.tensor_mul(semb[:], embt[:], sig[:])

            # silu(emb)^T -> sembT[e_part, e_tile, b]
            sembT = params.tile([128, E // 128, BL], F32)
            for et in range(E // 128):
                pt2 = psum_p.tile([128, BL], F32, tag="pp")
                nc.tensor.transpose(
                    pt2[:], semb[:, et * 128:(et + 1) * 128], ident[:BL, :BL])
                nc.vector.tensor_copy(sembT[:, et, :], pt2[:])

            # bias columns
            adab = params.tile([128, 2 * C // 128], F32)
            nc.sync.dma_start(
                adab[:], adab_d.ap().rearrange("(jt jp) -> jp jt", jp=128))
            convb = params.tile([128, KT], F32)
            nc.sync.dma_start(
                convb[:], convb_d.ap().rearrange("(ot op) -> op ot", op=128))

            # ada_w -> adaT (PE transpose) then
            # modT[jt][j_part, b] = ada_w^T-gemm(silu(emb)) + ada_b.
            # Scale rows (jt 2,3) first: they gate wpp and the first GEMM.
            adaw = prep.tile([128, 2 * C // 128, E], F32)
            nc.sync.dma_start(
                adaw[:], adaw_d.ap().rearrange("(jt jp) e -> jp jt e", jp=128))
            modT = []
            for jt in range(2 * C // 128):
                mt = params.tile([128, BL], F32, tag=f"modT{jt}")
                modT.append(mt)
            for jt in (2, 3, 0, 1):
                at = prep.tile([128, E // 128, 128], F32, tag=f"adaT{jt}")
                for et in range(E // 128):
                    pt = psum_p.tile([128, 128], F32, tag="pp")
                    nc.tensor.transpose(
                        pt[:], adaw[:, jt, et * 128:(et + 1) * 128], ident[:])
                    if et % 2 == 0:
                        nc.vector.tensor_copy(at[:, et, :], pt[:])
                    else:
                        nc.scalar.copy(at[:, et, :], pt[:])
                pmm = psum_p.tile([128, BL], F32, tag="pm")
                for et in range(E // 128):
                    nc.tensor.matmul(
                        pmm[:], at[:, et, :], sembT[:, et, :],
                        start=(et == 0), stop=(et == E // 128 - 1))
                nc.vector.tensor_scalar_add(
                    modT[jt][:], pmm[:], adab[:, jt:jt + 1])

            # lam*s bias rows for every sample: s = wT-gemm(shift) + conv_b,
            # stored transposed as [1, b, o, 128] bf16 for the K=1 bias MM
            lsrow = params.tile([1, BL, OT, 128], BF16)
            for b in range(BL):
                for o in range(OT):
                    ps = psum_p.tile([128, 1], F32, tag="ps")
                    for k in range(KT):
                        nc.tensor.matmul(
                            ps[:], wT[:, k, o * 128:(o + 1) * 128],
                            modT[k][:, b:b + 1],
                            start=(k == 0), stop=(k == KT - 1))
                    sco = prep.tile([128, 1], F32, tag=f"sco{b}_{o}")
                    nc.vector.tensor_scalar(
                        sco[:], ps[:], convb[:, o:o + 1], LAM,
                        Op.add, Op.mult)
                    pst = psum_p.tile([1, 128], F32, tag="pst")
                    nc.tensor.transpose(pst[:], sco[:], ident[:])
                    nc.vector.tensor_copy(lsrow[:, b, o, :], pst[:])

        # main-loop PSUM: pg pair tiles [128, 2, PW] f32 = 4 banks, 2 bufs
        psum_g = ctx.enter_context(
            tc.tile_pool(name="psum_g", bufs=2, space="PSUM"))

        # ---- main loop -------------------------------------------------
        for b in range(BL):
            # W''[c_part, k, o] = lam*(1+scale[c]) * wT   (bf16, DVE)
            sc = spool.tile([128, KT], F32, tag="sc")
            for k in range(KT):
                nc.vector.tensor_scalar(
                    sc[:, k:k + 1], modT[KT + k][:, b:b + 1],
                    1.0, LAM, Op.add, Op.mult)
            wpp = wpool.tile([128, KT, C], BF16, tag="wpp")
            for k in range(KT):
                nc.vector.tensor_scalar_mul(
                    wpp[:, k, :], wT[:, k, :], sc[:, k:k + 1])

            for j in range(NU):
                xj = xpool.tile([128, KT, PW], BF16, tag="xj")
                nc.sync.dma_start(
                    xj[:],
                    x_d.ap()[b].rearrange("(kt kp) w -> kp kt w", kp=128)
                    [:, :, bass.ts(j, PW)])

                # x^2: GPSIMD takes first GPX pixels per ktile, DVE the rest
                xsq = qpool.tile([128, KT, PW], BF16, tag="xsq")
                if GPX > 0:
                    nc.gpsimd.tensor_tensor(
                        xsq[:, :, :GPX], xj[:, :, :GPX], xj[:, :, :GPX],
                        Op.mult)
                if GPX < PW:
                    nc.vector.tensor_mul(
                        xsq[:, :, GPX:], xj[:, :, GPX:], xj[:, :, GPX:])

                # one PSUM pair-tile per unit: [:, 0, :] doubles as the pm
                # stats buffer before the main GEMM overwrites it.
                # Matmul outputs are split in 512-col halves (PSUM bank cap).
                pg = psum_g.tile([128, OT, 2, 512], F32, tag="pg")
                for h in range(2):
                    hs = bass.ts(h, 512)
                    for k in range(KT):
                        nc.tensor.matmul(pg[:, 0, h, :], onesC[:],
                                         xsq[:, k, hs],
                                         start=(k == 0), stop=(k == KT - 1))

                # rms = rsqrt(mean x^2): custom DVE op (first RMSD pixels)
                # and ACT ln+exp (rest) as a load-balance split
                rmsb = rpool.tile([128, PW], BF16, tag="rmsb")
                pm = pg[:, 0, :, :].rearrange("p h w -> p (h w)")
                if RMSD > 0:
                    nc.vector._custom_dve(
                        RSQRT_SEED_NR, out=rmsb[:, :RMSD],
                        in0=pm[:, :RMSD],
                        s0=_RSQ_A2, s1=_RSQ_B2, imm2=3.0)
                if RMSD < PW:
                    lnm = rpool.tile([128, PW - RMSD], F32, tag="lnm")
                    nc.scalar.activation(lnm[:], pm[:, RMSD:], AF.Ln,
                                         bias=eps_col[:], scale=1.0)
                    nc.scalar.activation(rmsb[:, RMSD:], lnm[:], AF.Exp,
                                         bias=zero_col[:], scale=-0.5)

                # xn = x * rms  (DVE TT bf16 2x, per ktile)
                xn = npool.tile([128, KT, PW], BF16, tag="xn")
                for k in range(KT):
                    nc.vector.tensor_mul(xn[:, k, :], xj[:, k, :], rmsb[:])

                # main GEMM + K=1 bias row: pg[:, o, :] = W'' xn + lam*s
                for o in range(OT):
                    for h in range(2):
                        hs = bass.ts(h, 512)
                        for k in range(KT):
                            nc.tensor.matmul(
                                pg[:, o, h, :],
                                wpp[:, k, o * 128:(o + 1) * 128],
                                xn[:, k, hs], start=(k == 0), stop=False)
                        nc.tensor.matmul(
                            pg[:, o, h, :], lsrow[:, b, o, :], ones_row[:],
                            start=False, stop=True)

                # es = Exp(pg/lam + ln(lam*alpha))  (ACT, one op per pair)
                es = epool.tile([128, OT, PW], BF16, tag="es")
                nc.scalar.activation(
                    es[:].rearrange("p o w -> p (o w)"),
                    pg[:].rearrange("p o h w -> p (o h w)"),
                    AF.Exp, bias=lnla_col[:], scale=1.0 / LAM)

                # y = min(es, lam*a) + max(pg - lam*a, -lam*a)  (custom DVE)
                yo = ypool.tile([128, OT, PW], BF16, tag="yo")
                nc.vector._custom_dve(
                    SELU_COMBINE,
                    out=yo[:].rearrange("p o w -> p (o w)"),
                    in0=es[:].rearrange("p o w -> p (o w)"),
                    in1=pg[:].rearrange("p o h w -> p (o h w)"),
                    s0=-LA, s1=-LA, imm2=LA)

                nc.sync.dma_start(
                    y_d.ap()[b].rearrange("(ot op) w -> op ot w", op=128)
                    [:, :, bass.ts(j, PW)],
                    yo[:])


_program_cache = None


def _get_program():
    global _program_cache
    if _program_cache is None:
        _program_cache = _build_program()
    return _program_cache


def kernel(x, emb, ada_w, ada_b, conv_w, conv_b):
    import ml_dtypes
    nc = _get_program()
    x = np.ascontiguousarray(
        np.asarray(x, dtype=np.float32).reshape(B, C, HW)
    ).astype(ml_dtypes.bfloat16)
    emb = np.ascontiguousarray(np.asarray(emb, dtype=np.float32))
    ada_w = np.ascontiguousarray(np.asarray(ada_w, dtype=np.float32))
    ada_b = np.ascontiguousarray(np.asarray(ada_b, dtype=np.float32))
    conv_w = np.ascontiguousarray(np.asarray(conv_w, dtype=np.float32))
    conv_b = np.ascontiguousarray(np.asarray(conv_b, dtype=np.float32))

    in_maps = []
    for c in range(NCORES):
        sl = slice(c * BL, (c + 1) * BL)
        in_maps.append({
            "x": np.ascontiguousarray(x[sl]),
            "emb": emb[sl],
            "ada_w": ada_w,
            "ada_b": ada_b,
            "conv_w": conv_w,
            "conv_b": conv_b,
        })

    res = bass_utils.run_bass_kernel_spmd(
        nc, in_maps, core_ids=list(range(NCORES)))
    y = np.concatenate(
        [np.asarray(r["y"]).astype(np.float32).reshape(BL, C, H, W)
         for r in res.results], axis=0)
    return y


# revision 15
# speedup vs baseline: 1.2968x; 1.2968x over previous
"""Trainium2 Bass kernel for nn_ConditionalChannelProjection (v2, bf16).

Reference computation (per sample b):
    mod = silu(emb) @ ada_w.T + ada_b          -> shift (C,), scale (C,)
    rms = rsqrt(mean_c(x^2) + eps)             -> per-pixel over channels
    xm  = (x * rms) * (1 + scale) + shift
    y   = selu(conv_w @ xm + conv_b)           (1x1 conv == channel GEMM)

v2 design (per 1024-pixel unit, bf16 data paths, fp32 PSUM):
    xsq  = x*x                                  (GPSIMD/DVE pixel-split)
    pm   = ones(1/C)-matmul(xsq)                (PE -> PSUM, bcast over parts)
    rmsb = rsqrt~(pm)                           (custom DVE op: linear seed +
                                                 1 Newton step, single pass;
                                                 partially ACT ln/exp as a
                                                 load-balance knob)
    xn   = x * rmsb                             (DVE TT bf16 2x)
    pg   = W''-gemm(xn) + lam*s (K=1 bias row)  (PE, bf16, fp32 PSUM)
      where W''[c,o] = lam*(1+scale_c)*w[o,c],  s[o] = w@shift + conv_b
    es   = Exp(pg/lam + ln(lam*alpha))          (ACT, one op per o-pair)
    y    = min(es, lam*a) + max(pg - lam*a, -lam*a)   (custom DVE op, 1 pass)
      == selu(u)*...: pg = lam*u, es = lam*a*e^u; exact selu identity.

Sharding: data-parallel over batch, 4 samples per core, params replicated.
I/O in bf16 (host converts); rel-err budget 2e-2, measured ~4e-3.
"""

import numpy as np

import concourse.bass as bass
import concourse.bacc as bacc
import concourse.tile as tile
import concourse.mybir as mybir
from concourse import bass_utils
from concourse.masks import make_identity
from concourse.alu_op_type import AluOpType as Op

# ---------------------------------------------------------------------------
# ACT table pinning: both Exp and Ln resolve to natural_log_exp_and_others so
# a single ACT_TABLE_LOAD is emitted (the greedy insertion pass would
# otherwise thrash between exp_and_others and natural_log sets).
import concourse.bacc as _bacc_mod
import concourse.hw_specs as _hw_specs

_ORIG_GET_TABLES = _hw_specs.get_activation_tables
_KEEP_TABLE = "natural_log_exp_and_others"


def _patched_get_tables(arch):
    tables = _ORIG_GET_TABLES(arch)
    return {name: (funcs if name == _KEEP_TABLE else set())
            for name, funcs in tables.items()}


_bacc_mod.get_activation_tables = _patched_get_tables

# ---------------------------------------------------------------------------
# Custom DVE ops (registered into concourse.dve_ops' module-level registry,
# which bass_utils/bass2jax read by object identity).
from concourse import dve_ops as _dve_ops_mod
from concourse.dve_ops import DveOp, get_dve_sub_opcode
from concourse.dve_spec import (
    C0, C1, C2, Spec, Src0, Src1, lower, _has_src1, maxx, minn, sq,
)
from concourse.dve_uop import DveOpSpec


def _register_dve_op(name, spec, subdim=False):
    for existing in _dve_ops_mod.OPS:
        if existing.name == name:
            return existing
    row = max(_dve_ops_mod._SUB_OPCODE_FOR_NAME.values()) + 1
    assert row < 0x20, "custom-DVE opcode rows exhausted"
    _dve_ops_mod._SUB_OPCODE_FOR_NAME[name] = row
    shas = {}
    for ver in ("v3", "v4"):
        try:
            s = DveOpSpec(name=name, opcode=row, uops=lower(spec, ver=ver),
                          rd1_en=_has_src1(spec))
            shas[ver] = s.sha(ver)
        except Exception:
            pass
    op = DveOp(name, spec, subdim=subdim, uops_sha=shas)
    _dve_ops_mod.OPS.append(op)
    _dve_ops_mod.CUSTOM_DVE_SPECS[name] = op.spec
    return op


# rsqrt(m) ~= z*(3 - m*(2z)^2), z = C0 - C1*m: distribution-weighted linear
# seed + one Newton-Raphson step. Fit on the empirical mean-square range of
# randn data (m in [0.55, 2.3], bulk near 1): rms rel err 4.5e-4.
_RSQ_A2 = 0.69984783  # a/2
_RSQ_B2 = 0.19598216  # b/2
_z = C0 - C1 * Src0
RSQRT_SEED_NR = _register_dve_op(
    "RSQRT_SEED_NR_ANT",
    Spec(
        body=_z * (C2 - Src0 * sq(_z + _z)),
        reference=lambda in0, in1, c0, c1, c2:
            (c0 - c1 * in0) * (c2 - in0 * ((c0 - c1 * in0) * 2.0) ** 2),
    ),
)

# selu combine: out = min(es, imm2) + max(pg + s0, s1)
# with s0 = -lam*alpha, s1 = -lam*alpha, imm2 = lam*alpha.
SELU_COMBINE = _register_dve_op(
    "SELU_COMBINE_ANT",
    Spec(
        body=minn(Src0, C2) + maxx(Src1 + C0, C1),
        reference=lambda in0, in1, s0, s1, imm2:
            np.minimum(in0, imm2) + np.maximum(in1 + s0, s1),
    ),
)

AF = mybir.ActivationFunctionType
F32 = mybir.dt.float32
BF16 = mybir.dt.bfloat16

B, C, H, W, E = 32, 256, 64, 64, 1024
HW = H * W                    # 4096 pixels per sample
NCORES = 8
BL = B // NCORES              # 4 samples per core
PW = 1024                     # pixels per unit
NU = HW // PW                 # 4 units per sample
KT = C // 128                 # 2 channel tiles
OT = C // 128                 # 2 output tiles
EPS = 1e-6

LAM = 1.0507009873554804934193349852946
ALPHA = 1.6732632423543772848170429916717
LA = LAM * ALPHA
LN_LA = float(np.log(LA))

# load-balance knobs
GPX = 832      # pixels per ktile of x^2 done on GPSIMD (rest on DVE)
RMSD = 0       # pixels of rms via custom-DVE rsqrt (rest via ACT ln+exp)


def _build_program(reps=1):
    nc = bacc.Bacc("TRN2", target_bir_lowering=False, debug=False,
                   num_devices=NCORES)

    x_d = nc.dram_tensor("x", (BL, C, HW), BF16, kind="ExternalInput")
    emb_d = nc.dram_tensor("emb", (BL, E), F32, kind="ExternalInput")
    adaw_d = nc.dram_tensor("ada_w", (2 * C, E), F32, kind="ExternalInput")
    adab_d = nc.dram_tensor("ada_b", (2 * C,), F32, kind="ExternalInput")
    convw_d = nc.dram_tensor("conv_w", (C, C), F32, kind="ExternalInput")
    convb_d = nc.dram_tensor("conv_b", (C,), F32, kind="ExternalInput")
    y_d = nc.dram_tensor("y", (BL, C, HW), BF16, kind="ExternalOutput")

    with tile.TileContext(nc) as tc:
        for _ in range(reps):
            _kernel(nc, tc, x_d, emb_d, adaw_d, adab_d, convw_d, convb_d, y_d)

    nc.compile()
    return nc


def _kernel(nc, tc, x_d, emb_d, adaw_d, adab_d, convw_d, convb_d, y_d):
    from contextlib import ExitStack
    ctx = ExitStack()
    with ctx:
        consts = ctx.enter_context(tc.tile_pool(name="consts", bufs=1))
        params = ctx.enter_context(tc.tile_pool(name="params", bufs=1))
        xpool = ctx.enter_context(tc.tile_pool(name="xpool", bufs=3))
        qpool = ctx.enter_context(tc.tile_pool(name="qpool", bufs=3))
        npool = ctx.enter_context(tc.tile_pool(name="npool", bufs=3))
        rpool = ctx.enter_context(tc.tile_pool(name="rpool", bufs=3))
        epool = ctx.enter_context(tc.tile_pool(name="epool", bufs=2))
        ypool = ctx.enter_context(tc.tile_pool(name="ypool", bufs=3))

        # ---- constants -------------------------------------------------
        ident = consts.tile([128, 128], F32)
        make_identity(nc, ident[:])
        onesC = consts.tile([128, 128], BF16)
        nc.gpsimd.memset(onesC[:], 1.0 / C)
        zero_col = consts.tile([128, 1], F32)
        nc.gpsimd.memset(zero_col[:], 0.0)
        eps_col = consts.tile([128, 1], F32)
        nc.gpsimd.memset(eps_col[:], EPS)

        # ---- parameter prep (runs once, overlapped with first x DMA) ---
        with tc.tile_pool(name="prep", bufs=1) as prep, \
             tc.tile_pool(name="psum_p", bufs=2, space="PSUM") as psum_p:

            # conv_w -> wT[c_part, k_tile, o]  (transposed via PE, fp32)
            cw = prep.tile([128, KT, C], F32)
            nc.sync.dma_start(
                cw[:], convw_d.ap().rearrange("(ot op) c -> op ot c", op=128))
            wT = params.tile([128, KT, C], F32)
            for k in range(KT):
                for m in range(KT):
                    pt = psum_p.tile([128, 128], F32, tag="pp")
                    nc.tensor.transpose(
                        pt[:], cw[:, m, k * 128:(k + 1) * 128], ident[:])
                    nc.vector.tensor_copy(wT[:, k, m * 128:(m + 1) * 128],
                                          pt[:])

            # silu(emb) via exp/reciprocal (keeps ACT table = ln/exp set)
            embt = prep.tile([BL, E], F32)
            nc.sync.dma_start(embt[:], emb_d.ap())
            sig = prep.tile([BL, E], F32)
            nc.scalar.activation(sig[:], embt[:], AF.Exp, bias=zero_col[:BL],
                                 scale=-1.0)
            nc.vector.tensor_scalar_add(sig[:], sig[:], 1.0)
            nc.vector.reciprocal(sig[:], sig[:])
            semb = prep.tile([BL, E], F32)
            nc.vector.tensor_mul(semb[:], embt[:], sig[:])

            # silu(emb)^T -> sembT[e_part, e_tile, b]
            sembT = params.tile([128, E // 128, BL], F32)
            for et in range(E // 128):
                pt2 = psum_p.tile([128, BL], F32, tag="pp")
                nc.tensor.transpose(
                    pt2[:], semb[:, et * 128:(et + 1) * 128], ident[:BL, :BL])
                nc.vector.tensor_copy(sembT[:, et, :], pt2[:])

            # bias columns
            adab = params.tile([128, 2 * C // 128], F32)
            nc.sync.dma_start(
                adab[:], adab_d.ap().rearrange("(jt jp) -> jp jt", jp=128))
            convb = params.tile([128, KT], F32)
            nc.sync.dma_start(
                convb[:], convb_d.ap().rearrange("(ot op) -> op ot", op=128))

            # ada_w -> adaT (PE transpose) then
            # modT[jt][j_part, b] = ada_w^T-gemm(silu(emb)) + ada_b.
            # Scale rows (jt 2,3) first: they gate wpp and the first GEMM.
            adaw = prep.tile([128, 2 * C // 128, E], F32)
            nc.sync.dma_start(
                adaw[:], adaw_d.ap().rearrange("(jt jp) e -> jp jt e", jp=128))
            modT = []
            for jt in range(2 * C // 128):
                mt = params.tile([128, BL], F32, tag=f"modT{jt}")
                modT.append(mt)
            for jt in (2, 3, 0, 1):
                at = prep.tile([128, E // 128, 128], F32, tag=f"adaT{jt}")
                for et in range(E // 128):
                    pt = psum_p.tile([128, 128], F32, tag="pp")
                    nc.tensor.transpose(
                        pt[:], adaw[:, jt, et * 128:(et + 1) * 128], ident[:])
                    if et % 2 == 0:
                        nc.vector.tensor_copy(at[:, et, :], pt[:])
                    else:
                        nc.scalar.copy(at[:, et, :], pt[:])
                pmm = psum_p.tile([128, BL], F32, tag="pm")
                for et in range(E // 128):
                    nc.tensor.matmul(
                        pmm[:], at[:, et, :], sembT[:, et, :],
                        start=(et == 0), stop=(et == E // 128 - 1))
                nc.vector.tensor_scalar_add(
                    modT[jt][:], pmm[:], adab[:, jt:jt + 1])

            # fused bias columns for every sample:
            #   s[b,o]  = wT-gemm(shift_b) + conv_b      (PE matvec)
            #   br[b,o] = lam*s - lam*alpha              (selu-combine scalar)
            #   be[b,o] = s + ln(lam*alpha)              (exp bias)
            br = params.tile([128, BL, OT], F32)
            be = params.tile([128, BL, OT], F32)
            for b in range(BL):
                for o in range(OT):
                    ps = psum_p.tile([128, 1], F32, tag="ps")
                    for k in range(KT):
                        nc.tensor.matmul(
                            ps[:], wT[:, k, o * 128:(o + 1) * 128],
                            modT[k][:, b:b + 1],
                            start=(k == 0), stop=(k == KT - 1))
                    sco = prep.tile([128, 1], F32, tag=f"sco{b}_{o}")
                    nc.vector.tensor_scalar_add(
                        sco[:], ps[:], convb[:, o:o + 1])
                    nc.vector.tensor_scalar(
                        br[:, b, o:o + 1], sco[:], LAM, -LA,
                        Op.mult, Op.add)
                    nc.vector.tensor_scalar_add(
                        be[:, b, o:o + 1], sco[:], LN_LA)

            # W''[c_part, k, o] = lam*(1+scale[c]) * wT  for every sample
            wpps = []
            for b in range(BL):
                sc = prep.tile([128, KT], F32, tag=f"scb{b}")
                for k in range(KT):
                    nc.vector.tensor_scalar(
                        sc[:, k:k + 1], modT[KT + k][:, b:b + 1],
                        1.0, LAM, Op.add, Op.mult)
                wpp = params.tile([128, KT, C], BF16, tag=f"wpp{b}")
                for k in range(KT):
                    nc.vector.tensor_scalar_mul(
                        wpp[:, k, :], wT[:, k, :], sc[:, k:k + 1])
                wpps.append(wpp)

        # main-loop PSUM: pg pair tiles [128, 2, 2, 512] f32 = 4 banks x 2
        psum_g = ctx.enter_context(
            tc.tile_pool(name="psum_g", bufs=2, space="PSUM"))

        # ---- main loop: software-pipelined over 16 units ---------------
        units = [(b, j) for b in range(BL) for j in range(NU)]
        state = {}

        def frontend(i):
            b, j = units[i]
            xj = xpool.tile([128, KT, PW], BF16, tag="xj")
            nc.sync.dma_start(
                xj[:],
                x_d.ap()[b].rearrange("(kt kp) w -> kp kt w", kp=128)
                [:, :, bass.ts(j, PW)])

            # x^2: GPSIMD takes first GPX pixels per ktile, DVE the rest
            xsq = qpool.tile([128, KT, PW], BF16, tag="xsq")
            if GPX > 0:
                nc.gpsimd.tensor_tensor(
                    xsq[:, :, :GPX], xj[:, :, :GPX], xj[:, :, :GPX],
                    Op.mult)
            if GPX < PW:
                nc.vector.tensor_mul(
                    xsq[:, :, GPX:], xj[:, :, GPX:], xj[:, :, GPX:])

            # PSUM pair-tile: [:, 0, :] doubles as the stats buffer before
            # the main GEMM overwrites it. Matmul outputs use 512-col
            # halves (PSUM bank cap).
            pg = psum_g.tile([128, OT, 2, 512], F32, tag="pg")
            for k in range(KT):
                for h in range(2):
                    nc.tensor.matmul(pg[:, 0, h, :], onesC[:],
                                     xsq[:, k, bass.ts(h, 512)],
                                     start=(k == 0), stop=(k == KT - 1))

            # rms = rsqrt(mean x^2): custom DVE op for the first RMSD
            # pixels, ACT ln+exp for the rest (load-balance split)
            rmsb = rpool.tile([128, PW], BF16, tag="rmsb")
            pm = pg[:, 0, :, :].rearrange("p h w -> p (h w)")
            if RMSD > 0:
                nc.vector._custom_dve(
                    RSQRT_SEED_NR, out=rmsb[:, :RMSD], in0=pm[:, :RMSD],
                    s0=_RSQ_A2, s1=_RSQ_B2, imm2=3.0)
            if RMSD < PW:
                lnm = rpool.tile([128, PW - RMSD], F32, tag="lnm")
                nc.scalar.activation(lnm[:], pm[:, RMSD:], AF.Ln,
                                     bias=eps_col[:], scale=1.0)
                nc.scalar.activation(rmsb[:, RMSD:], lnm[:], AF.Exp,
                                     bias=zero_col[:], scale=-0.5)

            # xn = x * rms  (DVE TT bf16 2x, per ktile)
            xn = npool.tile([128, KT, PW], BF16, tag="xn")
            for k in range(KT):
                nc.vector.tensor_mul(xn[:, k, :], xj[:, k, :], rmsb[:])
            state[i] = (pg, xn)

        def backend(i):
            b, j = units[i]
            pg, xn = state.pop(i)
            wpp = wpps[b]
            for o in range(OT):
                for k in range(KT):
                    for h in range(2):
                        nc.tensor.matmul(
                            pg[:, o, h, :],
                            wpp[:, k, o * 128:(o + 1) * 128],
                            xn[:, k, bass.ts(h, 512)],
                            start=(k == 0), stop=(k == KT - 1))

            es = epool.tile([128, OT, PW], BF16, tag="es")
            yo = ypool.tile([128, OT, PW], BF16, tag="yo")
            for o in range(OT):
                # es = Exp(pg/lam + be)  (ACT)
                nc.scalar.activation(
                    es[:, o, :], pg[:, o, :, :].rearrange("p h w -> p (h w)"),
                    AF.Exp, bias=be[:, b, o:o + 1], scale=1.0 / LAM)
                # y = min(es, lam*a) + max(pg + br, -lam*a)  (custom DVE)
                nc.vector._custom_dve(
                    SELU_COMBINE,
                    out=yo[:, o, :],
                    in0=es[:, o, :],
                    in1=pg[:, o, :, :].rearrange("p h w -> p (h w)"),
                    s0=br[:, b, o:o + 1], s1=-LA, imm2=LA)

            nc.sync.dma_start(
                y_d.ap()[b].rearrange("(ot op) w -> op ot w", op=128)
                [:, :, bass.ts(j, PW)],
                yo[:])

        frontend(0)
        for i in range(len(units)):
            if i + 1 < len(units):
                frontend(i + 1)
            backend(i)


_program_cache = None


def _get_program():
    global _program_cache
    if _program_cache is None:
        _program_cache = _build_program()
    return _program_cache


def kernel(x, emb, ada_w, ada_b, conv_w, conv_b):
    import ml_dtypes
    nc = _get_program()
    x = np.ascontiguousarray(
        np.asarray(x, dtype=np.float32).reshape(B, C, HW)
    ).astype(ml_dtypes.bfloat16)
    emb = np.ascontiguousarray(np.asarray(emb, dtype=np.float32))
    ada_w = np.ascontiguousarray(np.asarray(ada_w, dtype=np.float32))
    ada_b = np.ascontiguousarray(np.asarray(ada_b, dtype=np.float32))
    conv_w = np.ascontiguousarray(np.asarray(conv_w, dtype=np.float32))
    conv_b = np.ascontiguousarray(np.asarray(conv_b, dtype=np.float32))

    in_maps = []
    for c in range(NCORES):
        sl = slice(c * BL, (c + 1) * BL)
        in_maps.append({
            "x": np.ascontiguousarray(x[sl]),
            "emb": emb[sl],
            "ada_w": ada_w,
            "ada_b": ada_b,
            "conv_w": conv_w,
            "conv_b": conv_b,
        })

    res = bass_utils.run_bass_kernel_spmd(
        nc, in_maps, core_ids=list(range(NCORES)))
    y = np.concatenate(
        [np.asarray(r["y"]).astype(np.float32).reshape(BL, C, H, W)
         for r in res.results], axis=0)
    return y
